# revision 40
# baseline (speedup 1.0000x reference)
"""Trainium2 Bass kernel for AxisLengthNetMetric (chamfer-distance + L1-size metric).

Reference computation (per row n of N = 262144):
  gt_box row -> size (cols 3:6), rx (6:9), ry (9:12)
  rx_hat = rx/|rx|, ry_hat = ry/|ry|, rz = cross(rx_hat, ry_hat)
  corners u_c = sum_k sign[c,k] * 0.5*size[k] * axis_k   (8 corners, +-pairs)
  chamfer(corners, pred_pts[n]): d[p,q] = |a_p - b_q|^2, dist1 = min_q, dist2 = min_p
  out[0] = mean over (N,8) of dist1+dist2 ; out[1] = mean |size - pred_size|

Kernel strategy (v4):
- data parallel over 8 cores; per core 32768 rows as 128 partitions x 256.
- only 4 distinct corners up to sign: with u' = 2u (prescaled), g' = u'.b = 2 u.b,
    d[(i,+),q] = a2_i + (b2_q - g'),  d[(i,-),q] = a2_i + (b2_q + g'),
    dist2[q]   = b2_q + min_i(a2_i - |g'|).
- phase 0 computes the corner basis u' for ALL rows once (big ops, one
  reciprocal/sqrt, GPSIMD cross product); phase 1 loops 4 tiles of the heavy
  pairwise work (pred DMA, dot products, bf16 e/t tensors, bf16 min-trees).
- em/ep and min-trees in bf16 (2x DVE mode, contiguous-half pairwise mins;
  end-to-end rel err ~6e-6; validated in numpy emulation).
- no relu (clamp effect ~1e-9 here) and no post-adds: a2/b2 contributions are
  accumulated via ACT Square accum_out and recombined on the host:
  sum_cd = sum(mins) + 2*sum_i a2 + sum_q b2.
"""

import numpy as np

import concourse.bacc as bacc
import concourse.bass as bass  # noqa: F401
import concourse.tile as tile
from concourse import mybir

F32 = mybir.dt.float32
BF16 = mybir.dt.bfloat16
ALU = mybir.AluOpType
ACTF = mybir.ActivationFunctionType
AX = mybir.AxisListType

P = 128
N_CORES = 8
N_TOTAL = 262144
NC_N = N_TOTAL // N_CORES  # 32768 rows per core
G_PROD = 64                # rows per partition per heavy tile -> 4 tiles

# accT slots per tile
MINSUM, SQA, SQB, L1 = 0, 1, 2, 3
NSLOT = 4


def build_nc(nc_n=NC_N, G=G_PROD):
    GA = nc_n // P             # all rows per partition
    ntiles = GA // G
    assert ntiles * P * G == nc_n

    nc = bacc.Bacc("TRN2", target_bir_lowering=False, debug=False)

    gt = nc.dram_tensor("gt", [nc_n, 12], F32, kind="ExternalInput").ap()
    pred = nc.dram_tensor("pred", [nc_n, 24], F32, kind="ExternalInput").ap()
    ps = nc.dram_tensor("ps", [nc_n, 3], F32, kind="ExternalInput").ap()
    out = nc.dram_tensor("out", [P, ntiles * NSLOT], F32, kind="ExternalOutput").ap()

    gt_r = gt.rearrange("(p g) f -> p g f", p=P)
    pred_r = pred.rearrange("(p g) f -> p g f", p=P)
    ps_r = ps.rearrange("(p g) f -> p g f", p=P)

    with tile.TileContext(nc) as tc:
        with (
            tc.tile_pool(name="per", bufs=1) as per,   # persistent / phase-0
            tc.tile_pool(name="io", bufs=3) as io,
            tc.tile_pool(name="scr", bufs=1) as scr,
            tc.tile_pool(name="xe", bufs=2) as xe,     # cross-engine handoffs
        ):
            accT = per.tile([P, ntiles, NSLOT], F32)

            # warm the ACT function tables (Square/Sqrt/Abs/Identity) before
            # any data dependency, so LoadActFuncSet overlaps the first DMA
            warm = per.tile([P, 2], F32)
            nc.vector.memset(warm, 1.0)
            for fn in (ACTF.Square, ACTF.Sqrt, ACTF.Abs, ACTF.Identity):
                nc.scalar.activation(warm[:, 0:1], warm[:, 1:2], fn)

            # ================= phase 0: corner basis for all rows =============
            # chunked so coords compute starts after a fraction of the gt DMA
            gta = per.tile([P, GA, 12], F32)
            uta = per.tile([P, GA, 4, 3], F32)
            a2ba = per.tile([P, GA, 4], BF16)
            NCHUNK = ntiles
            GC = GA // NCHUNK  # per-chunk SQA accum slots line up
            for c in range(NCHUNK):
                cs = slice(c * GC, (c + 1) * GC)
                gtc = gta[:, cs]
                nc.sync.dma_start(out=gtc, in_=gt_r[:, cs])

                sqt = scr.tile([P, GC, 6], F32, tag="sqt")
                nc.scalar.square(sqt, gtc[:, :, 6:12])
                n2t = scr.tile([P, GC, 2], F32, tag="n2t")
                nc.vector.tensor_reduce(
                    n2t, sqt.rearrange("p g (v d) -> p g v d", d=3),
                    axis=AX.X, op=ALU.add,
                )
                srt = scr.tile([P, GC, 2], F32, tag="srt")
                nc.scalar.activation(srt, n2t, ACTF.Sqrt)  # |r|
                ivt = scr.tile([P, GC, 2], F32, tag="ivt")
                nc.vector.reciprocal(ivt, srt)             # 1/|r|
                c01t = scr.tile([P, GC, 2], F32, tag="c01t")  # sx/|rx|, sy/|ry|
                nc.vector.tensor_mul(c01t, gtc[:, :, 3:5], ivt)
                tzt = scr.tile([P, GC, 1], F32, tag="tzt")
                nc.vector.tensor_mul(tzt, ivt[:, :, 0:1], ivt[:, :, 1:2])
                czt = scr.tile([P, GC, 1], F32, tag="czt")    # sz/(|rx||ry|)
                nc.vector.tensor_mul(czt, gtc[:, :, 5:6], tzt)

                # cross product (raw rx x ry) on GPSIMD
                rxet = xe.tile([P, GC, 5], F32, tag="rxet")
                ryet = xe.tile([P, GC, 5], F32, tag="ryet")
                nc.scalar.copy(rxet[:, :, 0:3], gtc[:, :, 6:9])
                nc.scalar.copy(rxet[:, :, 3:5], gtc[:, :, 6:8])
                nc.scalar.copy(ryet[:, :, 0:3], gtc[:, :, 9:12])
                nc.scalar.copy(ryet[:, :, 3:5], gtc[:, :, 9:11])
                m1t = xe.tile([P, GC, 3], F32, tag="m1t")
                m2t = xe.tile([P, GC, 3], F32, tag="m2t")
                crt = xe.tile([P, GC, 3], F32, tag="crt")
                nc.gpsimd.tensor_mul(m1t, rxet[:, :, 1:4], ryet[:, :, 2:5])
                nc.gpsimd.tensor_mul(m2t, rxet[:, :, 2:5], ryet[:, :, 1:4])
                nc.gpsimd.tensor_sub(crt, m1t, m2t)

                v01t = scr.tile([P, GC, 2, 3], F32, tag="v01t")
                nc.vector.tensor_mul(
                    v01t,
                    gtc[:, :, 6:12].rearrange("p g (v d) -> p g v d", d=3),
                    c01t.unsqueeze(3).broadcast_to((P, GC, 2, 3)),
                )
                v2t = scr.tile([P, GC, 3], F32, tag="v2t")
                nc.vector.tensor_mul(v2t, crt, czt.broadcast_to((P, GC, 3)))
                wt = scr.tile([P, GC, 2, 3], F32, tag="wt")
                nc.vector.tensor_add(
                    wt[:, :, 0, :], v01t[:, :, 0, :], v01t[:, :, 1, :]
                )
                nc.vector.tensor_sub(
                    wt[:, :, 1, :], v01t[:, :, 0, :], v01t[:, :, 1, :]
                )
                utc = uta[:, cs]
                v2b = v2t.unsqueeze(2).broadcast_to((P, GC, 2, 3))
                nc.vector.tensor_add(utc[:, :, 0:2, :], wt, v2b)
                nc.vector.tensor_sub(utc[:, :, 2:4, :], wt, v2b)

            # a2: squares (ACT, accum -> per-chunk SQA slot), reduce to bf16.
            # Emitted after the coords loop so it fills engine gaps instead of
            # extending the phase-0 critical chain.
            for c in range(NCHUNK):
                cs = slice(c * GC, (c + 1) * GC)
                squt = xe.tile([P, GC, 4, 3], F32, tag="squt")
                nc.scalar.activation(
                    squt, uta[:, cs], ACTF.Square, scale=0.5,  # (u'/2)^2 = u^2
                    accum_out=accT[:, c, SQA : SQA + 1],
                )
                with nc.allow_low_precision("single bf16 round-off, same as cast"):
                    nc.vector.tensor_reduce(
                        a2ba[:, cs], squt, axis=AX.X, op=ALU.add
                    )

            # ================= phase 1: pairwise chamfer per tile =============
            for t in range(ntiles):
                sl = slice(t * G, (t + 1) * G)
                ut = uta[:, sl]
                bt = io.tile([P, G, 8, 3], F32, tag="pred")
                pst = io.tile([P, G, 3], F32, tag="ps")
                nc.sync.dma_start(
                    out=bt, in_=pred_r[:, sl].rearrange("p g (q d) -> p g q d", d=3)
                )
                nc.sync.dma_start(out=pst, in_=ps_r[:, sl])

                sqbt = xe.tile([P, G, 8, 3], F32, tag="sqbt")
                nc.scalar.activation(
                    sqbt, bt, ACTF.Square, accum_out=accT[:, t, SQB : SQB + 1]
                )

                def ue(d):
                    return ut[:, :, :, d].unsqueeze(3).broadcast_to((P, G, 4, 8))

                def be(d):
                    return bt[:, :, :, d].unsqueeze(2).broadcast_to((P, G, 4, 8))

                m0g = scr.tile([P, G, 4, 8], F32, tag="m0g")
                m1g = xe.tile([P, G, 4, 8], F32, tag="m1g")
                m2g = xe.tile([P, G, 4, 8], F32, tag="m2g")
                gb = xe.tile([P, G, 4, 8], BF16, tag="gb")
                if t == 0:
                    # first tile: split so the pipeline fills without waiting
                    # on a full (slower) GPSIMD burst
                    nc.gpsimd.tensor_mul(m2g, ue(2), be(2))
                    nc.vector.tensor_mul(m1g, ue(1), be(1))
                    nc.vector.tensor_add(m1g, m1g, m2g)
                else:
                    nc.gpsimd.tensor_mul(m1g, ue(1), be(1))
                    nc.gpsimd.tensor_mul(m2g, ue(2), be(2))
                    nc.gpsimd.tensor_add(m1g, m1g, m2g)
                nc.vector.tensor_mul(m0g, ue(0), be(0))
                nc.vector.tensor_add(gb, m0g, m1g)  # bf16 out: g' = 2 u.b
                # |g'| written q-major-transposed so the t2 path runs in 2x mode
                agbT = xe.tile([P, G, 8, 4], BF16, tag="agbT")
                nc.scalar.activation(agbT.transpose([0, 1, 3, 2]), gb, ACTF.Abs)

                # b2 reduced straight to bf16 (fp32 accumulate inside DVE)
                a2b = a2ba[:, sl]
                b2b = scr.tile([P, G, 8], BF16, tag="b2b")
                with nc.allow_low_precision("single bf16 round-off, same as cast"):
                    nc.vector.tensor_reduce(b2b, sqbt, axis=AX.X, op=ALU.add)

                # ---- em/ep/t2 in bf16 (2x mode) ------------------------------
                # em and ep share one (G, 8, 8) tile: rows 0:4 = em_i, 4:8 = ep_i,
                # so their q-min-trees run as single combined ops.
                b2bc = b2b.unsqueeze(2).broadcast_to((P, G, 4, 8))
                a2bcT = a2b.unsqueeze(2).broadcast_to((P, G, 8, 4))
                eeb = scr.tile([P, G, 8, 8], BF16, tag="eeb")
                t2b = scr.tile([P, G, 8, 4], BF16, tag="t2b")
                nc.vector.tensor_sub(eeb[:, :, 0:4, :], b2bc, gb)
                nc.vector.tensor_add(eeb[:, :, 4:8, :], b2bc, gb)
                nc.vector.tensor_sub(t2b, a2bcT, agbT)

                # ---- min-trees (bf16 2x), results into St --------------------
                St = scr.tile([P, G, 16], BF16, tag="St")
                e1 = scr.tile([P, G, 8, 4], BF16, tag="e1")
                # ett packs the penultimate levels of both trees: [:,0]=e-pairs,
                # [:,1]=t2-pairs, so one final min writes all of St
                ett = scr.tile([P, G, 2, 8, 2], BF16, tag="ett")
                nc.vector.tensor_tensor(
                    e1, eeb[:, :, :, 0:4], eeb[:, :, :, 4:8], op=ALU.min
                )
                nc.vector.tensor_tensor(
                    ett[:, :, 0], e1[:, :, :, 0:2], e1[:, :, :, 2:4], op=ALU.min
                )
                # t2b is (G, 8, 4) q-major: min over the innermost i axis
                nc.vector.tensor_tensor(
                    ett[:, :, 1], t2b[:, :, :, 0:2], t2b[:, :, :, 2:4], op=ALU.min
                )
                nc.vector.tensor_tensor(
                    St.rearrange("p g (x q) -> p g x q", x=2),
                    ett[:, :, :, :, 0], ett[:, :, :, :, 1], op=ALU.min
                )

                # ---- accumulate sums on ACT ----------------------------------
                junk16 = scr.tile([P, G, 16], BF16, tag="junk16")
                nc.scalar.activation(
                    junk16, St, ACTF.Identity, accum_out=accT[:, t, MINSUM : MINSUM + 1]
                )
                l1d = xe.tile([P, G, 3], F32, tag="l1d")
                nc.gpsimd.tensor_sub(l1d, pst, gta[:, sl, 3:6])
                junk3 = scr.tile([P, G, 3], F32, tag="junk3")
                nc.scalar.activation(
                    junk3, l1d, ACTF.Abs, accum_out=accT[:, t, L1 : L1 + 1]
                )

            nc.sync.dma_start(out=out, in_=accT.rearrange("p t x -> p (t x)"))

    nc.compile()
    return nc


_CACHE = {}


def _get_nc():
    if "nc" not in _CACHE:
        _CACHE["nc"] = build_nc()
    return _CACHE["nc"]


def combine_partials(outs):
    """outs: list of (P, ntiles*NSLOT) arrays -> (cd_sum, l1_sum) float64."""
    tot_min = 0.0
    tot_sqa = 0.0
    tot_sqb = 0.0
    tot_l1 = 0.0
    for o in outs:
        o = o.astype(np.float64).reshape(P, -1, NSLOT)
        tot_min += o[:, :, MINSUM].sum()
        tot_sqa += o[:, :, SQA].sum()
        tot_sqb += o[:, :, SQB].sum()
        tot_l1 += o[:, :, L1].sum()
    cd_sum = tot_min + 2.0 * tot_sqa + tot_sqb
    return cd_sum, tot_l1


def kernel(pred_pts, pred_size, gt_box):
    from concourse.bass_utils import run_bass_kernel_spmd

    pred_pts = np.asarray(pred_pts, dtype=np.float32)
    pred_size = np.asarray(pred_size, dtype=np.float32)
    gt_box = np.asarray(gt_box, dtype=np.float32)

    N = pred_pts.shape[0]
    assert N == N_TOTAL, f"expected {N_TOTAL} rows, got {N}"
    gt_flat = np.ascontiguousarray(gt_box.reshape(N, 12))
    pred = np.ascontiguousarray(pred_pts.reshape(N, 24))
    ps = np.ascontiguousarray(pred_size)

    in_maps = [
        {
            "gt": gt_flat[i * NC_N : (i + 1) * NC_N],
            "pred": pred[i * NC_N : (i + 1) * NC_N],
            "ps": ps[i * NC_N : (i + 1) * NC_N],
        }
        for i in range(N_CORES)
    ]
    res = run_bass_kernel_spmd(_get_nc(), in_maps, core_ids=list(range(N_CORES)))
    cd_sum, l1_sum = combine_partials([r["out"] for r in res.results])
    cd = cd_sum / (N * 8)
    l1 = l1_sum / (N * 3)
    return np.array([cd, l1], dtype=np.float32)
